# revision 62
# baseline (speedup 1.0000x reference)
"""ConvTransformerEncoderLayer on 8 trn2 NeuronCores.

Sharding: pure data-parallel over batch (B=8 -> 1 batch element per core).
Each core runs the full layer for its batch element; no collectives.

v8 layout strategy (S=1024, D=512, H=8, hd=64, DFF=2048):
  - ALL inputs merged into ONE dram tensor "blob" per core (per-call operand
    count 13 -> 2; the PJRT/axon dispatch path pays a per-operand cost that
    dominated the old per-call time). partition_id input dropped
    (enable_partition_id=False; no collectives).
  - score path (X, wq/wk/wv/wo, Q, K, AVT) float32r: self-loading weights,
    full PE rate at free-dim 512, fp32-accurate scores. Value/FFN path
    (VTx, et, w1, xT, hT, w2) bf16: halves SBUF + DMA bytes; PSUM
    accumulates fp32 everywhere, so rel err stays ~1e-3 (gate 2e-2).
  - Q,K convs produce [c, s]; V conv produces V^T [t, c] (+ ones column per
    head) so AV emits av^T [d, s] with the softmax denominator as a psum row.
  - scores of a head PAIR (bases 0/64) issue back-to-back as 64x128 row
    tiles (T0/T8) -> concurrent on the PE array, ~2x scores throughput.
  - softmax without max-subtraction (scores are O(10), fp32 exp safe).
  - attention software-pipelined at MATMUL granularity: each score chunk
    (2 row-tiled MMs + 2 exps) is interleaved with conv MMs of the NEXT
    ct-group and pending AV MMs, so PE never throttles to Act's exp rate
    waiting for score psum banks to drain; the last ct (no next conv)
    uses the first Wo/LN1 tiles as filler instead. Up to 2 pending units
    (4 et slots).
  - startup: X tiles alternate sync/scalar DMA queues ahead of all weights;
    wq/wk are ct_out-major so conv ct0 starts after 1/4 of the weight bytes;
    srcs has its own SBUF slot and streams on the gpsimd queue during
    attention; final AV units interleave with the first Wo/LN1 tiles.
  - LayerNorm normalize is one DVE tensor_scalar: (z-mu)*rstd; gamma/beta
    folded into W1/b1 host-side (device fixups only when nontrivial);
    FFN1 bias+relu on DVE (tensor_scalar add+max), not Act.
  - bo+Wo@bv folded into residual src host-side; b1+W1@be1 folded into b1;
    no bias matmuls anywhere.
  - SBUF slots are retagged across phases (X->xs, Q->xT, K->y, et->hT).
  - kernel() uses a cached jit(shard_map) executor (trace once per process).
"""
import sys

sys.path.insert(0, "/opt/trn_rl_repo")
import numpy as np

P = 128          # partitions
S = 1024         # sequence
D = 512          # d_model
H = 8            # heads
HD = 64          # head dim
DFF = 2048
KS = 3           # conv kernel size
EPS = 1e-5
NCORES = 8
CT = D // P      # 4 channel tiles
ST = S // P      # 8 sequence tiles
FT = DFF // P    # 16 ff tiles
SH = 512         # matmul free-dim chunk (= psum bank)
LAG = 2          # attention software-pipeline depth (pair units)

# blob layout: name -> (offset, length) in fp32 SLOTS per partition.
# Startup-critical regions first (DMA issue order follows blob order).
# w1/w2 are shipped bf16 (2 per fp32 slot); everything else fp32.
_BLOB_SPEC = [
    ("srcT", CT * S // 2),     # 2048 (bf16)
    ("wv", CT * D // 2),       # 1024 (bf16)
    ("wq", CT * KS * D // 2),  # 3072 (bf16)
    ("wk", CT * KS * D // 2),  # 3072 (bf16)
    ("bq", CT),
    ("bk", CT),
    ("ident", P),
    ("wo", CT * D),            # 2048
    ("src_sd", ST * D),        # 4096
    ("w1", CT * DFF // 2),     # 4096 (bf16)
    ("b1", FT),
    ("w2", FT * D // 2),       # 4096 (bf16)
]
_EXT_NAMES = ["g1r", "r1r", "g2r", "be2r"]  # appended when flags set

_CACHE = {}


def _blob_layout(flags):
    spec = list(_BLOB_SPEC)
    for name, fl in zip(_EXT_NAMES, flags):
        if fl:
            spec.append((name, D))
    off = {}
    pos = 0
    for name, ln in spec:
        off[name] = (pos, ln)
        pos += ln
    return off, pos


def _build_nc(flags):
    resid_mul, resid_add, out_mul, out_add = flags
    import concourse.tile as tile
    from concourse import bacc, mybir

    f32 = mybir.dt.float32
    f32r = mybir.dt.float32r
    bf16 = mybir.dt.bfloat16
    AF = mybir.ActivationFunctionType
    ALU = mybir.AluOpType

    nc = bacc.Bacc("TRN2", target_bir_lowering=False, debug=False,
                   enable_asserts=False, num_devices=NCORES,
                   enable_partition_id=False)

    off, total = _blob_layout(flags)
    blob = nc.dram_tensor("blob", [P, total], f32r, kind="ExternalInput").ap()

    def bsl(name, *shape, dt=None):
        o, ln = off[name]
        ap = blob[:, o:o + ln]
        if shape:
            dims = dict(zip("abc", shape))
            pat = " ".join("abc"[:len(shape)]) + " rest"
            ap = ap.rearrange(f"p ({pat}) -> p " + " ".join("abc"[:len(shape)])
                              + " rest", **dims)
        if dt is not None:
            ap = ap.bitcast(dt)
        return ap

    def bslb(name, *shape):
        """bf16-packed region: bitcast FIRST (doubles free dim), then shape."""
        o, ln = off[name]
        ap = blob[:, o:o + ln].bitcast(bf16)
        if shape:
            dims = dict(zip("abc", shape))
            pat = " ".join("abc"[:len(shape)]) + " rest"
            ap = ap.rearrange(f"p ({pat}) -> p " + " ".join("abc"[:len(shape)])
                              + " rest", **dims)
        return ap

    out_d = nc.dram_tensor("out", [P, ST, D], f32, kind="ExternalOutput").ap()

    with tile.TileContext(nc) as tc:
        with (
            tc.tile_pool(name="big", bufs=1) as big,
            tc.tile_pool(name="etp", bufs=4) as etp,
            tc.tile_pool(name="small", bufs=1) as small,
            tc.tile_pool(name="tmp", bufs=1) as tmp,
            tc.tile_pool(name="tiny", bufs=4) as tiny,
            tc.tile_pool(name="nrm", bufs=1) as nrm,
            tc.tile_pool(name="psp", bufs=8, space="PSUM") as psp,
        ):
            # ---------- small constants (gpsimd queue, tiny) ----------
            identity = small.tile([P, P], f32r, tag="ident")
            nc.gpsimd.dma_start(identity[:], bsl("ident"))
            bq_t = small.tile([P, CT], f32, tag="bq")
            nc.gpsimd.dma_start(bq_t[:], bsl("bq", dt=f32))
            bk_t = small.tile([P, CT], f32, tag="bk")
            nc.gpsimd.dma_start(bk_t[:], bsl("bk", dt=f32))
            b1_t = small.tile([P, FT], f32, tag="b1")
            nc.gpsimd.dma_start(b1_t[:], bsl("b1", dt=f32))
            ext_t = {}
            for k, fl in zip(_EXT_NAMES, flags):
                if not fl:
                    continue
                ext_t[k] = small.tile([P, D], f32, tag=k)
                nc.gpsimd.dma_start(ext_t[k][:], bsl(k, dt=f32))
            epsv = small.tile([P, 1], f32, tag="eps")
            nc.vector.memset(epsv[:], EPS)

            # ---------- bulk DMAs: startup-critical first ----------
            # X tiles alternate sync/scalar queues so no weight DMA can cut
            # ahead of the src data; conv weights are ct_out-major so each
            # ct-group's slice lands just in time for its convs.
            X = big.tile([P, CT, S + 2], bf16, tag="A", name="X")
            nc.vector.memset(X[:, :, 0:1], 0.0)
            nc.vector.memset(X[:, :, S + 1:S + 2], 0.0)
            srcT_v = bslb("srcT", CT)
            wv = big.tile([P, CT, D], bf16, tag="WC", name="wv_s")
            wv_v = bslb("wv", CT)
            nc.sync.dma_start(X[:, 0, 1:S + 1], srcT_v[:, 0, :])
            nc.scalar.dma_start(X[:, 1, 1:S + 1], srcT_v[:, 1, :])
            nc.sync.dma_start(wv[:, 0:2], wv_v[:, 0:2])
            nc.scalar.dma_start(wv[:, 2:4], wv_v[:, 2:4])
            nc.sync.dma_start(X[:, 2, 1:S + 1], srcT_v[:, 2, :])
            nc.scalar.dma_start(X[:, 3, 1:S + 1], srcT_v[:, 3, :])
            # wq/wk blob layout: [P, ct_out, ci_t, k, 128]
            wq = big.tile([P, CT, CT, KS, P], bf16, tag="WA", name="wq_s")
            wk = big.tile([P, CT, CT, KS, P], bf16, tag="WB", name="wk_s")
            wq_v = bslb("wq", CT, CT, KS)
            wk_v = bslb("wk", CT, CT, KS)
            for ct in range(CT):
                nc.sync.dma_start(wq[:, ct], wq_v[:, ct])
                nc.scalar.dma_start(wk[:, ct], wk_v[:, ct])

            Q = big.tile([P, CT, S], bf16, tag="Q", name="Q")
            K = big.tile([P, CT, S], bf16, tag="K", name="K")
            VTx = big.tile([P, ST, H, HD + 1], bf16, tag="V", name="VTx")
            AVT = big.tile([P, CT, S], f32r, tag="AVT", name="AVT")
            # srcs has its OWN slot so its DMA isn't gated on VTx's death;
            # it streams in on the idle gpsimd queue during attention.
            srcs = big.tile([P, ST, D], f32, tag="SR", name="srcs")
            srcs_v = bsl("src_sd", ST, dt=f32)
            for half in range(2):
                nc.gpsimd.dma_start(srcs[:, 4 * half:4 * (half + 1)],
                                    srcs_v[:, 4 * half:4 * (half + 1)])

            # ---------- V conv -> VTx (V^T with a ones column per head) -----
            nc.vector.memset(VTx[:, :, :, HD:HD + 1], 1.0)
            for tt in range(ST):
                ps = psp.tile([P, SH], f32, tag="ps", bufs=6, name="psv")
                for ci in range(CT):
                    nc.tensor.matmul(ps[:], X[:, ci, 1 + tt * P:1 + (tt + 1) * P],
                                     wv[:, ci, :],
                                     start=(ci == 0), stop=(ci == CT - 1))
                nc.vector.tensor_copy(VTx[:, tt, :, 0:HD],
                                      ps.rearrange("p (h e) -> p h e", h=H))

            # wo into wv's slot (wv dead after V conv)
            wo = big.tile([P, CT, D], f32r, tag="WC", name="wo_s")
            nc.sync.dma_start(wo[:], bsl("wo", CT))

            # ---------- attention, software-pipelined at MM granularity ----
            # During a scores pair, PE can issue 2 score MMs (~430ns) per
            # 2 Act exps (~1.2us) that drain their psum banks -> PE would run
            # at Act's rate.  So each score chunk is interleaved with conv
            # MMs of the NEXT ct (Act-independent) and pending AV MMs.
            def gen_conv(dst, w, bias_t, ct):
                """Yields thunks: 6x (2 conv MMs) + 1 bias-add, per sc."""
                for sc in range(2):
                    ps = psp.tile([P, SH], f32, tag="ps", bufs=6, name="psqk")
                    seq = [(ci, k) for ci in range(CT) for k in range(KS)]
                    for j0 in range(0, len(seq), 2):
                        def emit_mm(chunk=seq[j0:j0 + 2], ps=ps, sc=sc, j0=j0):
                            for idx, (ci, k) in enumerate(chunk):
                                nc.tensor.matmul(
                                    ps[:], w[:, ct, ci, k, :],
                                    X[:, ci, sc * SH + k: sc * SH + k + SH],
                                    start=(j0 + idx == 0),
                                    stop=(j0 + idx == len(seq) - 1))
                        yield emit_mm
                    def emit_bias(ps=ps, sc=sc):
                        nc.vector.tensor_scalar_add(
                            dst[:, ct, sc * SH:(sc + 1) * SH], ps[:],
                            bias_t[:, ct:ct + 1])
                    yield emit_bias

            def gen_scores(ct, sc, ets):
                """Yields 8 thunks; each: both heads' 64x128 row-tiled score
                MMs for one tt (adjacent -> concurrent on T0/T8) + exps."""
                for tt in range(ST):
                    def emit(tt=tt):
                        for i in range(2):
                            base = HD * i
                            ps = psp.tile([P, SH], f32, tag="ps", bufs=6,
                                          name="pssc")
                            nc.tensor.matmul(
                                ps[:],
                                K[base:base + HD, ct, tt * P:(tt + 1) * P],
                                Q[base:base + HD, ct, sc * SH:(sc + 1) * SH],
                                start=True, stop=True)
                            nc.scalar.activation(ets[i][:, tt, :], ps[:],
                                                 AF.Exp, bias=0.0,
                                                 scale=1.0 / HD)
                    yield emit

            def gen_av(h, sc, et):
                """Yields 4x (2 AV MMs) + 1 normalize thunk."""
                avps = psp.tile([P, SH], f32, tag="ps", bufs=6, name="avps")
                for tt0 in range(0, ST, 2):
                    def emit_mm(tt0=tt0, avps=avps):
                        for tt in (tt0, tt0 + 1):
                            nc.tensor.matmul(avps[0:HD + 1, :],
                                             VTx[:, tt, h, :], et[:, tt, :],
                                             start=(tt == 0),
                                             stop=(tt == ST - 1))
                    yield emit_mm
                def emit_norm(avps=avps):
                    rrec = nrm.tile([1, SH], f32r, tag="rrec", name="rrec")
                    with nc.allow_low_precision(reason="f32r softmax denom"):
                        nc.vector.reciprocal(rrec[0:1, :], avps[HD:HD + 1, :])
                    rrep = nrm.tile([HD, SH], f32r, tag="rrep", name="rrep")
                    nc.gpsimd.partition_broadcast(rrep[:], rrec[0:1, :])
                    base_o = HD * (h % 2)
                    nc.vector.tensor_tensor(
                        out=AVT[base_o:base_o + HD, h // 2,
                                sc * SH:(sc + 1) * SH],
                        in0=avps[0:HD, :], in1=rrep[:], op=ALU.mult)
                yield emit_norm

            def av_unit(h, sc, et):
                for th in gen_av(h, sc, et):
                    th()

            # FFN-phase tiles + Wo/LN1 defined up-front so wo_ln1 can be
            # used as PE filler during the last ct's scores (which have no
            # next-ct conv to hide the Act exp backlog behind).
            xs = big.tile([P, ST, D], f32r, tag="A", name="xs")   # X's slot
            xT = big.tile([P, CT, S], bf16, tag="Q", name="xT")   # Q's slot
            y = big.tile([P, ST, D], f32, tag="K", name="y")      # K's slot

            def ln_prep(z):
                """-> (mv, rstd) for DVE normalize of z [P, D]."""
                stats = tiny.tile([P, 6], f32, tag="st6", name="st6")
                nc.vector.bn_stats(stats[:], z[:])
                mv = tiny.tile([P, 2], f32, tag="mv", name="mv")
                nc.vector.bn_aggr(mv[:], stats[:])
                sd = tiny.tile([P, 1], f32, tag="sd", name="sd")
                nc.scalar.activation(sd[:], mv[:, 1:2], AF.Sqrt,
                                     bias=epsv[:], scale=1.0)
                rstd = tiny.tile([P, 1], f32, tag="rstd", name="rstd")
                nc.vector.reciprocal(rstd[:], sd[:])
                return mv, rstd

            def wo_ln1(st):
                ps = psp.tile([P, SH], f32, tag="ps", bufs=6, name="pswo")
                for dt in range(CT):
                    nc.tensor.matmul(ps[:], AVT[:, dt, st * P:(st + 1) * P],
                                     wo[:, dt, :], start=(dt == 0),
                                     stop=(dt == CT - 1))
                z = tmp.tile([P, D], f32, tag="t1", bufs=1, name="z1")
                nc.vector.tensor_tensor(out=z[:], in0=ps[:],
                                        in1=srcs[:, st, :], op=ALU.add)
                mv, rstd = ln_prep(z)
                nc.vector.tensor_scalar(out=xs[:, st, :], in0=z[:],
                                        scalar1=mv[:, 0:1], scalar2=rstd[:],
                                        op0=ALU.subtract, op1=ALU.mult)

            def transpose_tile(st, dt, use_dve=False):
                tp = psp.tile([P, P], f32r, tag="tp", bufs=2, name="tp")
                nc.tensor.transpose(tp[:], xs[:, st, dt * P:(dt + 1) * P],
                                    identity[:])
                if use_dve:
                    # keep the psum->xT copy OFF the Act FIFO while exps are
                    # still draining (an Act copy would delay every later exp)
                    nc.vector.tensor_copy(xT[:, dt, st * P:(st + 1) * P],
                                          tp[:])
                else:
                    nc.scalar.activation(xT[:, dt, st * P:(st + 1) * P],
                                         tp[:], AF.Copy)

            wo_done = set()
            tr_done = set()

            # ct0 convs have nothing to hide behind (V conv precedes them)
            for th in gen_conv(Q, wq, bq_t, 0):
                th()
            for th in gen_conv(K, wk, bk_t, 0):
                th()

            pending = []      # (h, sc, et) units with no AV issued yet
            av_open = []      # in-flight av generators, oldest first

            def av_step():
                """Advance the oldest in-flight AV generator by one thunk,
                auto-starting the next pending unit when one finishes.
                Returns False ONLY when no AV work remains anywhere (so a
                False return makes it safe to emit work that depends on all
                prior units, e.g. the Wo/LN1 filler)."""
                while True:
                    if not av_open:
                        if not pending:
                            return False
                        av_open.append(iter(gen_av(*pending.pop(0))))
                    try:
                        next(av_open[0])()
                        return True
                    except StopIteration:
                        av_open.pop(0)

            for ct in range(CT):
                conv_next = []
                if ct + 1 < CT:
                    conv_next = list(gen_conv(Q, wq, bq_t, ct + 1)) + \
                                list(gen_conv(K, wk, bk_t, ct + 1))
                cn = iter(conv_next)
                for sc in range(2):
                    # ET pool has 4 slots: finish AV of all but <=2 live
                    # units before allocating this pair's 2 et tiles.
                    while len(pending) + len(av_open) > 2:
                        if not av_step():
                            break
                    filler = []
                    if ct == CT - 1 and sc == 1:
                        # last pair: no next-ct conv; once this ct's sc=0
                        # AVs drain (mid-loop), Wo/LN1 tiles become legal
                        # PE filler for the Act exp backlog.
                        def _mk_wo(st):
                            def f():
                                wo_ln1(st)
                                wo_done.add(st)
                            return f
                        def _mk_tr(st, dt):
                            def f():
                                transpose_tile(st, dt, use_dve=True)
                                tr_done.add((st, dt))
                            return f
                        filler = [_mk_wo(st) for st in range(ST // 2)] + \
                                 [_mk_tr(st, dt) for st in range(ST // 2)
                                  for dt in range(CT)]
                    ets = [etp.tile([P, ST, SH], bf16, tag="ET",
                                    name=f"et{2 * ct + i}_{sc}")
                           for i in range(2)]
                    for schunk in gen_scores(ct, sc, ets):
                        schunk()
                        # pace filler 1 conv + 1 AV per chunk so the conv
                        # stream lasts through BOTH sc phases; top up with a
                        # second conv chunk (or wo/transpose filler) only
                        # when AV work is dry.
                        did = 0
                        th = next(cn, None)
                        if th is not None:
                            th()
                            did += 1
                        if av_step():
                            did += 1
                        else:
                            th = next(cn, None)
                            if th is not None:
                                th()
                                did += 1
                        if did == 0 and filler:
                            filler.pop(0)()
                    for i in range(2):
                        pending.append((2 * ct + i, sc, ets[i]))
                # drain any leftover conv chunks of ct+1
                for th in cn:
                    th()
            # close out in-flight AV generators ONLY (leave never-started
            # units in `pending` for the tail, where they interleave with
            # the remaining Wo/LN1 tiles).
            while av_open:
                try:
                    next(av_open[0])()
                except StopIteration:
                    av_open.pop(0)
            final_units = list(pending)
            pending = None

            # FFN weights (bf16) into the dead conv-weight slots
            w1 = big.tile([P, CT, DFF], bf16, tag="WA", name="w1_s")
            o1, l1 = off["w1"]
            nc.scalar.dma_start(w1[:], blob[:, o1:o1 + l1].bitcast(bf16)
                                .rearrange("p (a rest) -> p a rest", a=CT))
            w2 = big.tile([P, FT, D], bf16, tag="WB", name="w2_s")
            o2, l2 = off["w2"]
            nc.sync.dma_start(w2[:], blob[:, o2:o2 + l2].bitcast(bf16)
                              .rearrange("p (a rest) -> p a rest", a=FT))
            # Tail: the last pair's exps are still draining on Act, so the
            # final AV units stall.  Materialize their thunks and pace them
            # through the FFN1 sc0 loop below; first finish any sc0 Wo/LN1
            # and transposes the filler didn't consume (they are
            # exp-independent and FFN1 sc0 needs the transposes).
            av_tail = [th for u in final_units for th in gen_av(*u)]
            for st in range(ST // 2):
                if st not in wo_done:
                    wo_ln1(st)
            for st in range(ST // 2):
                for dt in range(CT):
                    if (st, dt) not in tr_done:
                        transpose_tile(st, dt)
            # legal only after ALL sc1 AVs (av_tail) have been emitted:
            post_av = []
            for st in range(ST // 2, ST):
                def _wo(st=st):
                    wo_ln1(st)
                post_av.append(_wo)
                for dt in range(CT):
                    def _tr(st=st, dt=dt):
                        transpose_tile(st, dt)
                    post_av.append(_tr)

            # ---------- FFN, overlapped with second-half Wo/LN1 ----------
            def ffn1_tile(sc, hT, ft):
                ps = psp.tile([P, SH], f32, tag="ps", bufs=6, name="psf1")
                for dt in range(CT):
                    nc.tensor.matmul(ps[:], w1[:, dt, ft * P:(ft + 1) * P],
                                     xT[:, dt, sc * SH:(sc + 1) * SH],
                                     start=(dt == 0), stop=(dt == CT - 1))
                nc.vector.tensor_scalar(out=hT[ft // 8][:, ft % 8, :],
                                        in0=ps[:],
                                        scalar1=b1_t[:, ft:ft + 1],
                                        scalar2=0.0,
                                        op0=ALU.add, op1=ALU.max)

            def ffn2_tile(sc, hT, j):
                st = sc * (ST // 2) + j
                ps = psp.tile([P, SH], f32, tag="ps", bufs=6, name="psf2")
                for ft in range(FT):
                    nc.tensor.matmul(
                        ps[:], hT[ft // 8][:, ft % 8, j * P:(j + 1) * P],
                        w2[:, ft, :], start=(ft == 0), stop=(ft == FT - 1))
                if resid_mul or resid_add:
                    xr = tmp.tile([P, D], f32, tag="xr", bufs=1, name="xr")
                    cur = xs[:, st, :]
                    if resid_mul:
                        nc.vector.tensor_tensor(out=xr[:], in0=cur,
                                                in1=ext_t["g1r"][:],
                                                op=ALU.mult)
                        cur = xr[:]
                    if resid_add:
                        nc.vector.tensor_tensor(out=xr[:], in0=cur,
                                                in1=ext_t["r1r"][:],
                                                op=ALU.add)
                    resid_ap = xr[:]
                else:
                    resid_ap = xs[:, st, :]
                z = tmp.tile([P, D], f32, tag="t1", bufs=1, name="z2")
                nc.vector.tensor_tensor(out=z[:], in0=ps[:],
                                        in1=resid_ap, op=ALU.add)
                mv, rstd = ln_prep(z)
                if out_mul or out_add:
                    yt = tmp.tile([P, D], f32, tag="t2", bufs=1, name="yt")
                    nc.vector.tensor_scalar(out=yt[:], in0=z[:],
                                            scalar1=mv[:, 0:1],
                                            scalar2=rstd[:],
                                            op0=ALU.subtract, op1=ALU.mult)
                    cur = yt[:]
                    if out_mul:
                        nc.vector.tensor_tensor(out=y[:, st, :], in0=cur,
                                                in1=ext_t["g2r"][:],
                                                op=ALU.mult)
                        cur = y[:, st, :]
                    if out_add:
                        nc.vector.tensor_tensor(out=y[:, st, :], in0=cur,
                                                in1=ext_t["be2r"][:],
                                                op=ALU.add)
                else:
                    nc.vector.tensor_scalar(out=y[:, st, :], in0=z[:],
                                            scalar1=mv[:, 0:1],
                                            scalar2=rstd[:],
                                            op0=ALU.subtract, op1=ALU.mult)
                nc.sync.dma_start(out_d[:, st, :], y[:, st, :])

            hT0 = [etp.tile([P, FT // 2, SH], bf16, tag="ET",
                            name=f"hT0_{i}") for i in range(2)]
            # interleave: final AV units (paced ahead of each tile so their
            # psum banks free in program order), then second-half Wo/LN1 +
            # transposes, between first-half FFN1 tiles
            for ft in range(FT):
                if av_tail:
                    av_tail.pop(0)()
                else:
                    for _ in range(4):
                        if post_av:
                            post_av.pop(0)()
                ffn1_tile(0, hT0, ft)
            for th in av_tail + post_av:
                th()
            av_tail = post_av = None
            hT1 = [etp.tile([P, FT // 2, SH], bf16, tag="ET",
                            name=f"hT1_{i}") for i in range(2)]
            # interleave: second-half FFN1 between first-half FFN2 tiles
            for j in range(ST // 2):
                ffn2_tile(0, hT0, j)
                for k in range(4):
                    ffn1_tile(1, hT1, 4 * j + k)
            for j in range(ST // 2):
                ffn2_tile(1, hT1, j)

    nc.compile()
    return nc


def _bf16_pack(a):
    """fp32 array [P, ...] -> bf16 bytes viewed as fp32 [P, n/2]."""
    import ml_dtypes
    b = np.ascontiguousarray(a.astype(ml_dtypes.bfloat16))
    return b.reshape(P, -1).view(np.uint16).view(np.float32)


def _prep_inputs(src, Wq, bq, Wk, bk, Wv, bv, Wo, bo, W1, b1, W2, b2,
                 g1, be1, g2, be2):
    f = np.float32

    def ctile(w):  # [co, ci(, k)] conv weight -> [p, ci_t(, k), co]
        wt = np.ascontiguousarray(np.moveaxis(w, 0, -1))  # [ci(,k), co]
        return np.ascontiguousarray(
            wt.reshape(CT, P, *wt.shape[1:]).transpose(1, 0, *range(2, wt.ndim + 1)))

    W1f = (W1 * np.asarray(g1)[None, :]).astype(f)      # fold gamma1
    b1f = (b1 + W1 @ be1).astype(f)                     # fold beta1
    r1 = (be1 + b2).astype(f)                           # residual additive fix
    flags = (not np.allclose(g1, 1.0), not np.allclose(r1, 0.0),
             not np.allclose(g2, 1.0), not np.allclose(be2, 0.0))

    def ctile_o(w):  # [co, ci, k] -> [P, co_t, ci_t, k, 128] (ct_out-major)
        a = ctile(w)                                     # [P, CT, KS, D]
        return np.ascontiguousarray(
            a.reshape(P, CT, KS, CT, P).transpose(0, 3, 1, 2, 4))

    pieces = {
        "wq": _bf16_pack(ctile_o(Wq)),                   # [P, CT, CT, KS, 128]
        "wk": _bf16_pack(ctile_o(Wk)),
        "wv": _bf16_pack(ctile(Wv[:, :, 0])),            # [P, CT, D]
        "wo": np.ascontiguousarray(
            Wo.T.reshape(CT, P, D).transpose(1, 0, 2)).astype(f),
        "w1": _bf16_pack(np.ascontiguousarray(
            W1f.T.reshape(CT, P, DFF).transpose(1, 0, 2))),
        "w2": _bf16_pack(np.ascontiguousarray(
            W2.T.reshape(FT, P, D).transpose(1, 0, 2))),
        "bq": np.ascontiguousarray(bq.reshape(CT, P).T).astype(f),
        "bk": np.ascontiguousarray(bk.reshape(CT, P).T).astype(f),
        "b1": np.ascontiguousarray(b1f.reshape(FT, P).T).astype(f),
        "ident": np.eye(P, dtype=f),
    }
    if flags[0]:
        pieces["g1r"] = np.ascontiguousarray(np.broadcast_to(g1, (P, D))).astype(f)
    if flags[1]:
        pieces["r1r"] = np.ascontiguousarray(np.broadcast_to(r1, (P, D))).astype(f)
    if flags[2]:
        pieces["g2r"] = np.ascontiguousarray(np.broadcast_to(g2, (P, D))).astype(f)
    if flags[3]:
        pieces["be2r"] = np.ascontiguousarray(np.broadcast_to(be2, (P, D))).astype(f)

    off, total = _blob_layout(flags)
    shared = np.zeros((P, total), f)
    for name, (o, ln) in off.items():
        if name in ("srcT", "src_sd"):
            continue
        shared[:, o:o + ln] = pieces[name].reshape(P, ln)

    bo2 = (bo + Wo @ bv).astype(f)                       # folded into residual
    o_srcT, l_srcT = off["srcT"]
    o_ssd, l_ssd = off["src_sd"]
    in_maps = []
    for b in range(NCORES):
        m = shared.copy()
        m[:, o_srcT:o_srcT + l_srcT] = _bf16_pack(np.ascontiguousarray(
            src[b].T.reshape(CT, P, S).transpose(1, 0, 2)))
        m[:, o_ssd:o_ssd + l_ssd] = np.ascontiguousarray(
            (src[b] + bo2[None, :]).reshape(ST, P, D).transpose(1, 0, 2)
        ).astype(f).reshape(P, l_ssd)
        in_maps.append({"blob": m})
    return in_maps, flags


def get_nc(flags=(False, False, False, False)):
    if ("nc", flags) not in _CACHE:
        _CACHE[("nc", flags)] = _build_nc(flags)
    return _CACHE[("nc", flags)]


def _get_runner(nc):
    """Cached jit(shard_map(bass_exec)) executor: trace/compile once, then
    each kernel() call is device_put + execute (run_bass_kernel_spmd
    rebuilds its jit closure every call, paying a full re-trace)."""
    key = ("runner", id(nc))
    if key in _CACHE:
        return _CACHE[key]
    import jax
    from jax.sharding import Mesh, PartitionSpec, NamedSharding
    try:
        from jax.shard_map import shard_map
    except ImportError:
        from jax.experimental.shard_map import shard_map
    from concourse import bass2jax, mybir

    bass2jax.install_neuronx_cc_hook()
    out_shape = (P, ST, D)
    out_avals = (jax.core.ShapedArray(out_shape, np.float32),)

    def _body(blob_in, out_zero):
        outs = bass2jax._bass_exec_p.bind(
            blob_in, out_zero,
            out_avals=out_avals,
            in_names=("blob", "out"),
            out_names=("out",),
            lowering_input_output_aliases=(),
            sim_require_finite=True,
            sim_require_nnan=True,
            nc=nc,
        )
        return tuple(outs)

    devices = jax.devices()[:NCORES]
    mesh = Mesh(np.asarray(devices), ("core",))
    spec = PartitionSpec("core")
    f = jax.jit(
        shard_map(_body, mesh=mesh, in_specs=(spec, spec), out_specs=(spec,),
                  check_rep=False),
        keep_unused=True,
    )
    sharding = NamedSharding(mesh, spec)
    zeros = np.zeros((NCORES * P, ST, D), np.float32)
    _CACHE[key] = (f, sharding, zeros, jax)
    return _CACHE[key]


def kernel(**inputs):
    in_maps, flags = _prep_inputs(**{k: np.asarray(v) for k, v in inputs.items()})
    nc = get_nc(flags)
    try:
        f, sharding, zeros, jax = _get_runner(nc)
        blob_all = np.concatenate([m["blob"] for m in in_maps], axis=0)
        blob_dev = jax.device_put(blob_all, sharding)
        out_dev = jax.device_put(zeros, sharding)
        (out_all,) = f(blob_dev, out_dev)
        out_np = np.asarray(out_all).reshape(NCORES, P, ST, D)
        outs = [out_np[c].transpose(1, 0, 2).reshape(S, D)
                for c in range(NCORES)]
        return np.stack(outs).astype(np.float32)
    except Exception:
        from concourse.bass_utils import run_bass_kernel_spmd
        res = run_bass_kernel_spmd(nc, in_maps, core_ids=list(range(NCORES)))
        outs = [r["out"].transpose(1, 0, 2).reshape(S, D) for r in res.results]
        return np.stack(outs).astype(np.float32)


# revision 64
# speedup vs baseline: 1.7896x; 1.7896x over previous
"""ConvTransformerEncoderLayer on 8 trn2 NeuronCores.

Sharding: pure data-parallel over batch (B=8 -> 1 batch element per core).
Each core runs the full layer for its batch element; no collectives.

v8 layout strategy (S=1024, D=512, H=8, hd=64, DFF=2048):
  - ALL inputs merged into ONE dram tensor "blob" per core (per-call operand
    count 13 -> 2; the PJRT/axon dispatch path pays a per-operand cost that
    dominated the old per-call time). partition_id input dropped
    (enable_partition_id=False; no collectives).
  - score path (X, wq/wk/wv/wo, Q, K, AVT) float32r: self-loading weights,
    full PE rate at free-dim 512, fp32-accurate scores. Value/FFN path
    (VTx, et, w1, xT, hT, w2) bf16: halves SBUF + DMA bytes; PSUM
    accumulates fp32 everywhere, so rel err stays ~1e-3 (gate 2e-2).
  - Q,K convs produce [c, s]; V conv produces V^T [t, c] (+ ones column per
    head) so AV emits av^T [d, s] with the softmax denominator as a psum row.
  - scores of a head PAIR (bases 0/64) issue back-to-back as 64x128 row
    tiles (T0/T8) -> concurrent on the PE array, ~2x scores throughput.
  - softmax without max-subtraction (scores are O(10), fp32 exp safe).
  - attention software-pipelined at MATMUL granularity: each score chunk
    (2 row-tiled MMs + 2 exps) is interleaved with conv MMs of the NEXT
    ct-group and pending AV MMs, so PE never throttles to Act's exp rate
    waiting for score psum banks to drain; the last ct (no next conv)
    uses the first Wo/LN1 tiles as filler instead. Up to 2 pending units
    (4 et slots).
  - startup: X tiles alternate sync/scalar DMA queues ahead of all weights;
    wq/wk are ct_out-major so conv ct0 starts after 1/4 of the weight bytes;
    srcs has its own SBUF slot and streams on the gpsimd queue during
    attention; final AV units interleave with the first Wo/LN1 tiles.
  - LayerNorm normalize is one DVE tensor_scalar: (z-mu)*rstd; gamma/beta
    folded into W1/b1 host-side (device fixups only when nontrivial);
    FFN1 bias+relu on DVE (tensor_scalar add+max), not Act.
  - bo+Wo@bv folded into residual src host-side; b1+W1@be1 folded into b1;
    no bias matmuls anywhere.
  - SBUF slots are retagged across phases (X->xs, Q->xT, K->y, et->hT).
  - kernel() uses a cached jit(shard_map) executor (trace once per process).
"""
import sys

sys.path.insert(0, "/opt/trn_rl_repo")
import numpy as np

P = 128          # partitions
S = 1024         # sequence
D = 512          # d_model
H = 8            # heads
HD = 64          # head dim
DFF = 2048
KS = 3           # conv kernel size
EPS = 1e-5
NCORES = 8
CT = D // P      # 4 channel tiles
ST = S // P      # 8 sequence tiles
FT = DFF // P    # 16 ff tiles
SH = 512         # matmul free-dim chunk (= psum bank)
LAG = 2          # attention software-pipeline depth (pair units)

# blob layout: name -> (offset, length) in fp32 SLOTS per partition.
# Startup-critical regions first (DMA issue order follows blob order).
# w1/w2 are shipped bf16 (2 per fp32 slot); everything else fp32.
_BLOB_SPEC = [
    ("srcT", CT * S // 2),     # 2048 (bf16)
    ("wv", CT * D // 2),       # 1024 (bf16)
    ("wq", CT * KS * D // 2),  # 3072 (bf16)
    ("wk", CT * KS * D // 2),  # 3072 (bf16)
    ("bq", CT),
    ("bk", CT),
    ("ident", P),
    ("wo", CT * D),            # 2048
    ("src_sd", ST * D),        # 4096
    ("w1", CT * DFF // 2),     # 4096 (bf16)
    ("b1", FT),
    ("w2", FT * D // 2),       # 4096 (bf16)
]
_EXT_NAMES = ["g1r", "r1r", "g2r", "be2r"]  # appended when flags set

_CACHE = {}


def _blob_layout(flags):
    spec = list(_BLOB_SPEC)
    for name, fl in zip(_EXT_NAMES, flags):
        if fl:
            spec.append((name, D))
    off = {}
    pos = 0
    for name, ln in spec:
        off[name] = (pos, ln)
        pos += ln
    return off, pos


def _build_nc(flags):
    resid_mul, resid_add, out_mul, out_add = flags
    import concourse.tile as tile
    from concourse import bacc, mybir

    f32 = mybir.dt.float32
    f32r = mybir.dt.float32r
    bf16 = mybir.dt.bfloat16
    AF = mybir.ActivationFunctionType
    ALU = mybir.AluOpType

    nc = bacc.Bacc("TRN2", target_bir_lowering=False, debug=False,
                   enable_asserts=False, num_devices=NCORES,
                   enable_partition_id=False)

    off, total = _blob_layout(flags)
    blob = nc.dram_tensor("blob", [P, total], f32r, kind="ExternalInput").ap()

    def bsl(name, *shape, dt=None):
        o, ln = off[name]
        ap = blob[:, o:o + ln]
        if shape:
            dims = dict(zip("abc", shape))
            pat = " ".join("abc"[:len(shape)]) + " rest"
            ap = ap.rearrange(f"p ({pat}) -> p " + " ".join("abc"[:len(shape)])
                              + " rest", **dims)
        if dt is not None:
            ap = ap.bitcast(dt)
        return ap

    def bslb(name, *shape):
        """bf16-packed region: bitcast FIRST (doubles free dim), then shape."""
        o, ln = off[name]
        ap = blob[:, o:o + ln].bitcast(bf16)
        if shape:
            dims = dict(zip("abc", shape))
            pat = " ".join("abc"[:len(shape)]) + " rest"
            ap = ap.rearrange(f"p ({pat}) -> p " + " ".join("abc"[:len(shape)])
                              + " rest", **dims)
        return ap

    out_d = nc.dram_tensor("out", [P, ST, D], f32, kind="ExternalOutput").ap()

    with tile.TileContext(nc) as tc:
        with (
            tc.tile_pool(name="big", bufs=1) as big,
            tc.tile_pool(name="etp", bufs=4) as etp,
            tc.tile_pool(name="small", bufs=1) as small,
            tc.tile_pool(name="tmp", bufs=1) as tmp,
            tc.tile_pool(name="tiny", bufs=4) as tiny,
            tc.tile_pool(name="nrm", bufs=1) as nrm,
            tc.tile_pool(name="psp", bufs=8, space="PSUM") as psp,
        ):
            # ---------- small constants (gpsimd queue, tiny) ----------
            identity = small.tile([P, P], f32r, tag="ident")
            nc.gpsimd.dma_start(identity[:], bsl("ident"))
            bq_t = small.tile([P, CT], f32, tag="bq")
            nc.gpsimd.dma_start(bq_t[:], bsl("bq", dt=f32))
            bk_t = small.tile([P, CT], f32, tag="bk")
            nc.gpsimd.dma_start(bk_t[:], bsl("bk", dt=f32))
            b1_t = small.tile([P, FT], f32, tag="b1")
            nc.gpsimd.dma_start(b1_t[:], bsl("b1", dt=f32))
            ext_t = {}
            for k, fl in zip(_EXT_NAMES, flags):
                if not fl:
                    continue
                ext_t[k] = small.tile([P, D], f32, tag=k)
                nc.gpsimd.dma_start(ext_t[k][:], bsl(k, dt=f32))
            epsv = small.tile([P, 1], f32, tag="eps")
            nc.vector.memset(epsv[:], EPS)

            # ---------- bulk DMAs: startup-critical first ----------
            # X tiles alternate sync/scalar queues so no weight DMA can cut
            # ahead of the src data; conv weights are ct_out-major so each
            # ct-group's slice lands just in time for its convs.
            X = big.tile([P, CT, S + 2], bf16, tag="A", name="X")
            nc.vector.memset(X[:, :, 0:1], 0.0)
            nc.vector.memset(X[:, :, S + 1:S + 2], 0.0)
            srcT_v = bslb("srcT", CT)
            wv = big.tile([P, CT, D], bf16, tag="WC", name="wv_s")
            wv_v = bslb("wv", CT)
            nc.sync.dma_start(X[:, 0, 1:S + 1], srcT_v[:, 0, :])
            nc.scalar.dma_start(X[:, 1, 1:S + 1], srcT_v[:, 1, :])
            nc.sync.dma_start(wv[:, 0:2], wv_v[:, 0:2])
            nc.scalar.dma_start(wv[:, 2:4], wv_v[:, 2:4])
            nc.sync.dma_start(X[:, 2, 1:S + 1], srcT_v[:, 2, :])
            nc.scalar.dma_start(X[:, 3, 1:S + 1], srcT_v[:, 3, :])
            # wq/wk blob layout: [P, ct_out, ci_t, k, 128]
            wq = big.tile([P, CT, CT, KS, P], bf16, tag="WA", name="wq_s")
            wk = big.tile([P, CT, CT, KS, P], bf16, tag="WB", name="wk_s")
            wq_v = bslb("wq", CT, CT, KS)
            wk_v = bslb("wk", CT, CT, KS)
            for ct in range(CT):
                nc.sync.dma_start(wq[:, ct], wq_v[:, ct])
                nc.scalar.dma_start(wk[:, ct], wk_v[:, ct])

            Q = big.tile([P, CT, S], bf16, tag="Q", name="Q")
            K = big.tile([P, CT, S], bf16, tag="K", name="K")
            VTx = big.tile([P, ST, H, HD + 1], bf16, tag="V", name="VTx")
            AVT = big.tile([P, CT, S], f32r, tag="AVT", name="AVT")
            # srcs has its OWN slot so its DMA isn't gated on VTx's death;
            # it streams in on the idle gpsimd queue during attention.
            srcs = big.tile([P, ST, D], f32, tag="SR", name="srcs")
            srcs_v = bsl("src_sd", ST, dt=f32)
            for half in range(2):
                nc.gpsimd.dma_start(srcs[:, 4 * half:4 * (half + 1)],
                                    srcs_v[:, 4 * half:4 * (half + 1)])

            # ---------- V conv -> VTx (V^T with a ones column per head) -----
            nc.vector.memset(VTx[:, :, :, HD:HD + 1], 1.0)
            for tt in range(ST):
                ps = psp.tile([P, SH], f32, tag="ps", bufs=6, name="psv")
                for ci in range(CT):
                    nc.tensor.matmul(ps[:], X[:, ci, 1 + tt * P:1 + (tt + 1) * P],
                                     wv[:, ci, :],
                                     start=(ci == 0), stop=(ci == CT - 1))
                nc.vector.tensor_copy(VTx[:, tt, :, 0:HD],
                                      ps.rearrange("p (h e) -> p h e", h=H))

            # wo into wv's slot (wv dead after V conv)
            wo = big.tile([P, CT, D], f32r, tag="WC", name="wo_s")
            nc.sync.dma_start(wo[:], bsl("wo", CT))

            # ---------- attention, software-pipelined at MM granularity ----
            # During a scores pair, PE can issue 2 score MMs (~430ns) per
            # 2 Act exps (~1.2us) that drain their psum banks -> PE would run
            # at Act's rate.  So each score chunk is interleaved with conv
            # MMs of the NEXT ct (Act-independent) and pending AV MMs.
            def gen_conv(dst, w, bias_t, ct):
                """Yields thunks: 6x (2 conv MMs) + 1 bias-add, per sc."""
                for sc in range(2):
                    ps = psp.tile([P, SH], f32, tag="ps", bufs=6, name="psqk")
                    seq = [(ci, k) for ci in range(CT) for k in range(KS)]
                    for j0 in range(0, len(seq), 2):
                        def emit_mm(chunk=seq[j0:j0 + 2], ps=ps, sc=sc, j0=j0):
                            for idx, (ci, k) in enumerate(chunk):
                                nc.tensor.matmul(
                                    ps[:], w[:, ct, ci, k, :],
                                    X[:, ci, sc * SH + k: sc * SH + k + SH],
                                    start=(j0 + idx == 0),
                                    stop=(j0 + idx == len(seq) - 1))
                        yield emit_mm
                    def emit_bias(ps=ps, sc=sc):
                        nc.vector.tensor_scalar_add(
                            dst[:, ct, sc * SH:(sc + 1) * SH], ps[:],
                            bias_t[:, ct:ct + 1])
                    yield emit_bias

            def gen_scores(ct, sc, ets):
                """Yields 8 thunks; each: both heads' 64x128 row-tiled score
                MMs for one tt (adjacent -> concurrent on T0/T8) + exps."""
                for tt in range(ST):
                    def emit(tt=tt):
                        for i in range(2):
                            base = HD * i
                            ps = psp.tile([P, SH], f32, tag="ps", bufs=6,
                                          name="pssc")
                            nc.tensor.matmul(
                                ps[:],
                                K[base:base + HD, ct, tt * P:(tt + 1) * P],
                                Q[base:base + HD, ct, sc * SH:(sc + 1) * SH],
                                start=True, stop=True)
                            nc.scalar.activation(ets[i][:, tt, :], ps[:],
                                                 AF.Exp, bias=0.0,
                                                 scale=1.0 / HD)
                    yield emit

            def gen_av(h, sc, et):
                """Yields 4x (2 AV MMs) + 1 normalize thunk."""
                avps = psp.tile([P, SH], f32, tag="ps", bufs=6, name="avps")
                for tt0 in range(0, ST, 2):
                    def emit_mm(tt0=tt0, avps=avps):
                        for tt in (tt0, tt0 + 1):
                            nc.tensor.matmul(avps[0:HD + 1, :],
                                             VTx[:, tt, h, :], et[:, tt, :],
                                             start=(tt == 0),
                                             stop=(tt == ST - 1))
                    yield emit_mm
                def emit_norm(avps=avps):
                    rrec = nrm.tile([1, SH], f32r, tag="rrec", name="rrec")
                    with nc.allow_low_precision(reason="f32r softmax denom"):
                        nc.vector.reciprocal(rrec[0:1, :], avps[HD:HD + 1, :])
                    rrep = nrm.tile([HD, SH], f32r, tag="rrep", name="rrep")
                    nc.gpsimd.partition_broadcast(rrep[:], rrec[0:1, :])
                    base_o = HD * (h % 2)
                    nc.vector.tensor_tensor(
                        out=AVT[base_o:base_o + HD, h // 2,
                                sc * SH:(sc + 1) * SH],
                        in0=avps[0:HD, :], in1=rrep[:], op=ALU.mult)
                yield emit_norm

            def av_unit(h, sc, et):
                for th in gen_av(h, sc, et):
                    th()

            # FFN-phase tiles + Wo/LN1 defined up-front so wo_ln1 can be
            # used as PE filler during the last ct's scores (which have no
            # next-ct conv to hide the Act exp backlog behind).
            xs = big.tile([P, ST, D], f32r, tag="A", name="xs")   # X's slot
            xT = big.tile([P, CT, S], bf16, tag="Q", name="xT")   # Q's slot
            y = big.tile([P, ST, D], f32, tag="K", name="y")      # K's slot

            def ln_prep(z):
                """-> (mv, rstd) for DVE normalize of z [P, D]."""
                stats = tiny.tile([P, 6], f32, tag="st6", name="st6")
                nc.vector.bn_stats(stats[:], z[:])
                mv = tiny.tile([P, 2], f32, tag="mv", name="mv")
                nc.vector.bn_aggr(mv[:], stats[:])
                sd = tiny.tile([P, 1], f32, tag="sd", name="sd")
                nc.scalar.activation(sd[:], mv[:, 1:2], AF.Sqrt,
                                     bias=epsv[:], scale=1.0)
                rstd = tiny.tile([P, 1], f32, tag="rstd", name="rstd")
                nc.vector.reciprocal(rstd[:], sd[:])
                return mv, rstd

            def wo_ln1(st):
                ps = psp.tile([P, SH], f32, tag="ps", bufs=6, name="pswo")
                for dt in range(CT):
                    nc.tensor.matmul(ps[:], AVT[:, dt, st * P:(st + 1) * P],
                                     wo[:, dt, :], start=(dt == 0),
                                     stop=(dt == CT - 1))
                z = tmp.tile([P, D], f32, tag="t1", bufs=1, name="z1")
                nc.vector.tensor_tensor(out=z[:], in0=ps[:],
                                        in1=srcs[:, st, :], op=ALU.add)
                mv, rstd = ln_prep(z)
                nc.vector.tensor_scalar(out=xs[:, st, :], in0=z[:],
                                        scalar1=mv[:, 0:1], scalar2=rstd[:],
                                        op0=ALU.subtract, op1=ALU.mult)

            def transpose_tile(st, dt, use_dve=False):
                tp = psp.tile([P, P], f32r, tag="tp", bufs=2, name="tp")
                nc.tensor.transpose(tp[:], xs[:, st, dt * P:(dt + 1) * P],
                                    identity[:])
                if use_dve:
                    # keep the psum->xT copy OFF the Act FIFO while exps are
                    # still draining (an Act copy would delay every later exp)
                    nc.vector.tensor_copy(xT[:, dt, st * P:(st + 1) * P],
                                          tp[:])
                else:
                    nc.scalar.activation(xT[:, dt, st * P:(st + 1) * P],
                                         tp[:], AF.Copy)

            wo_done = set()
            tr_done = set()

            # ct0 convs have nothing to hide behind (V conv precedes them)
            for th in gen_conv(Q, wq, bq_t, 0):
                th()
            for th in gen_conv(K, wk, bk_t, 0):
                th()

            pending = []      # (h, sc, et) units with no AV issued yet
            av_open = []      # in-flight av generators, oldest first

            def av_step():
                """Advance the oldest in-flight AV generator by one thunk,
                auto-starting the next pending unit when one finishes.
                Returns False ONLY when no AV work remains anywhere (so a
                False return makes it safe to emit work that depends on all
                prior units, e.g. the Wo/LN1 filler)."""
                while True:
                    if not av_open:
                        if not pending:
                            return False
                        av_open.append(iter(gen_av(*pending.pop(0))))
                    try:
                        next(av_open[0])()
                        return True
                    except StopIteration:
                        av_open.pop(0)

            carry = []        # deferred Q-sc1 conv of the CURRENT ct
            for ct in range(CT):
                # Defer next ct's Q-sc1 conv half into that ct's own sc0
                # scores phase (sc0 scores never read Q's sc1 columns, but K
                # must be complete for any sc) so the last ct-groups keep PE
                # filler through the Act-limited stretches.
                conv_next = []
                if ct + 1 < CT:
                    gq = list(gen_conv(Q, wq, bq_t, ct + 1))
                    gk = list(gen_conv(K, wk, bk_t, ct + 1))
                    half = len(gq) // 2
                    conv_next = gq[:half] + gk
                    next_carry = gq[half:]
                else:
                    next_carry = []
                cy = iter(carry)
                cn = iter(conv_next)
                carry = next_carry
                for sc in range(2):
                    if sc == 1:
                        # this ct's sc1 pair reads Q's sc1 columns: force
                        # any unconsumed deferred conv first.
                        for th in cy:
                            th()
                    # ET pool has 4 slots: finish AV of all but <=2 live
                    # units before allocating this pair's 2 et tiles.
                    while len(pending) + len(av_open) > 2:
                        if not av_step():
                            break
                    filler = []
                    if ct == CT - 1 and sc == 1:
                        # last pair: no next-ct conv; once this ct's sc=0
                        # AVs drain (mid-loop), Wo/LN1 tiles become legal
                        # PE filler for the Act exp backlog.
                        def _mk_wo(st):
                            def f():
                                wo_ln1(st)
                                wo_done.add(st)
                            return f
                        def _mk_tr(st, dt):
                            def f():
                                transpose_tile(st, dt, use_dve=True)
                                tr_done.add((st, dt))
                            return f
                        filler = [_mk_wo(st) for st in range(ST // 2)] + \
                                 [_mk_tr(st, dt) for st in range(ST // 2)
                                  for dt in range(CT)]
                    ets = [etp.tile([P, ST, SH], bf16, tag="ET",
                                    name=f"et{2 * ct + i}_{sc}")
                           for i in range(2)]
                    for schunk in gen_scores(ct, sc, ets):
                        schunk()
                        # pace filler 1 conv + 1 AV per chunk so the conv
                        # stream lasts through BOTH sc phases; top up with a
                        # second conv chunk (or wo/transpose filler) only
                        # when AV work is dry.
                        did = 0
                        th = next(cy, None) or next(cn, None)
                        if th is not None:
                            th()
                            did += 1
                        if av_step():
                            did += 1
                        else:
                            th = next(cy, None) or next(cn, None)
                            if th is not None:
                                th()
                                did += 1
                        if did == 0 and filler:
                            filler.pop(0)()
                    for i in range(2):
                        pending.append((2 * ct + i, sc, ets[i]))
                # drain any leftover conv chunks of ct+1
                for th in cn:
                    th()
            # close out in-flight AV generators ONLY (leave never-started
            # units in `pending` for the tail, where they interleave with
            # the remaining Wo/LN1 tiles).
            while av_open:
                try:
                    next(av_open[0])()
                except StopIteration:
                    av_open.pop(0)
            final_units = list(pending)
            pending = None

            # FFN weights (bf16) into the dead conv-weight slots
            w1 = big.tile([P, CT, DFF], bf16, tag="WA", name="w1_s")
            o1, l1 = off["w1"]
            nc.scalar.dma_start(w1[:], blob[:, o1:o1 + l1].bitcast(bf16)
                                .rearrange("p (a rest) -> p a rest", a=CT))
            w2 = big.tile([P, FT, D], bf16, tag="WB", name="w2_s")
            o2, l2 = off["w2"]
            nc.sync.dma_start(w2[:], blob[:, o2:o2 + l2].bitcast(bf16)
                              .rearrange("p (a rest) -> p a rest", a=FT))
            # Tail: the last pair's exps are still draining on Act, so the
            # final AV units stall.  Materialize their thunks and pace them
            # through the FFN1 sc0 loop below; first finish any sc0 Wo/LN1
            # and transposes the filler didn't consume (they are
            # exp-independent and FFN1 sc0 needs the transposes).
            av_tail = [th for u in final_units for th in gen_av(*u)]
            for st in range(ST // 2):
                if st not in wo_done:
                    wo_ln1(st)
            for st in range(ST // 2):
                for dt in range(CT):
                    if (st, dt) not in tr_done:
                        transpose_tile(st, dt)
            # legal only after ALL sc1 AVs (av_tail) have been emitted:
            post_av = []
            for st in range(ST // 2, ST):
                def _wo(st=st):
                    wo_ln1(st)
                post_av.append(_wo)
                for dt in range(CT):
                    def _tr(st=st, dt=dt):
                        transpose_tile(st, dt)
                    post_av.append(_tr)

            # ---------- FFN, overlapped with second-half Wo/LN1 ----------
            def ffn1_tile(sc, hT, ft):
                ps = psp.tile([P, SH], f32, tag="ps", bufs=6, name="psf1")
                for dt in range(CT):
                    nc.tensor.matmul(ps[:], w1[:, dt, ft * P:(ft + 1) * P],
                                     xT[:, dt, sc * SH:(sc + 1) * SH],
                                     start=(dt == 0), stop=(dt == CT - 1))
                nc.vector.tensor_scalar(out=hT[ft // 8][:, ft % 8, :],
                                        in0=ps[:],
                                        scalar1=b1_t[:, ft:ft + 1],
                                        scalar2=0.0,
                                        op0=ALU.add, op1=ALU.max)

            def ffn2_tile(sc, hT, j):
                st = sc * (ST // 2) + j
                ps = psp.tile([P, SH], f32, tag="ps", bufs=6, name="psf2")
                for ft in range(FT):
                    nc.tensor.matmul(
                        ps[:], hT[ft // 8][:, ft % 8, j * P:(j + 1) * P],
                        w2[:, ft, :], start=(ft == 0), stop=(ft == FT - 1))
                if resid_mul or resid_add:
                    xr = tmp.tile([P, D], f32, tag="xr", bufs=1, name="xr")
                    cur = xs[:, st, :]
                    if resid_mul:
                        nc.vector.tensor_tensor(out=xr[:], in0=cur,
                                                in1=ext_t["g1r"][:],
                                                op=ALU.mult)
                        cur = xr[:]
                    if resid_add:
                        nc.vector.tensor_tensor(out=xr[:], in0=cur,
                                                in1=ext_t["r1r"][:],
                                                op=ALU.add)
                    resid_ap = xr[:]
                else:
                    resid_ap = xs[:, st, :]
                z = tmp.tile([P, D], f32, tag="t1", bufs=1, name="z2")
                nc.vector.tensor_tensor(out=z[:], in0=ps[:],
                                        in1=resid_ap, op=ALU.add)
                mv, rstd = ln_prep(z)
                if out_mul or out_add:
                    yt = tmp.tile([P, D], f32, tag="t2", bufs=1, name="yt")
                    nc.vector.tensor_scalar(out=yt[:], in0=z[:],
                                            scalar1=mv[:, 0:1],
                                            scalar2=rstd[:],
                                            op0=ALU.subtract, op1=ALU.mult)
                    cur = yt[:]
                    if out_mul:
                        nc.vector.tensor_tensor(out=y[:, st, :], in0=cur,
                                                in1=ext_t["g2r"][:],
                                                op=ALU.mult)
                        cur = y[:, st, :]
                    if out_add:
                        nc.vector.tensor_tensor(out=y[:, st, :], in0=cur,
                                                in1=ext_t["be2r"][:],
                                                op=ALU.add)
                else:
                    nc.vector.tensor_scalar(out=y[:, st, :], in0=z[:],
                                            scalar1=mv[:, 0:1],
                                            scalar2=rstd[:],
                                            op0=ALU.subtract, op1=ALU.mult)
                nc.sync.dma_start(out_d[:, st, :], y[:, st, :])

            hT0 = [etp.tile([P, FT // 2, SH], bf16, tag="ET",
                            name=f"hT0_{i}") for i in range(2)]
            # interleave: final AV units (paced ahead of each tile so their
            # psum banks free in program order), then second-half Wo/LN1 +
            # transposes, between first-half FFN1 tiles
            for ft in range(FT):
                if av_tail:
                    av_tail.pop(0)()
                else:
                    for _ in range(4):
                        if post_av:
                            post_av.pop(0)()
                ffn1_tile(0, hT0, ft)
            for th in av_tail + post_av:
                th()
            av_tail = post_av = None
            hT1 = [etp.tile([P, FT // 2, SH], bf16, tag="ET",
                            name=f"hT1_{i}") for i in range(2)]
            # interleave: second-half FFN1 between first-half FFN2 tiles
            for j in range(ST // 2):
                ffn2_tile(0, hT0, j)
                for k in range(4):
                    ffn1_tile(1, hT1, 4 * j + k)
            for j in range(ST // 2):
                ffn2_tile(1, hT1, j)

    nc.compile()
    return nc


def _bf16_pack(a):
    """fp32 array [P, ...] -> bf16 bytes viewed as fp32 [P, n/2]."""
    import ml_dtypes
    b = np.ascontiguousarray(a.astype(ml_dtypes.bfloat16))
    return b.reshape(P, -1).view(np.uint16).view(np.float32)


def _prep_inputs(src, Wq, bq, Wk, bk, Wv, bv, Wo, bo, W1, b1, W2, b2,
                 g1, be1, g2, be2):
    f = np.float32

    def ctile(w):  # [co, ci(, k)] conv weight -> [p, ci_t(, k), co]
        wt = np.ascontiguousarray(np.moveaxis(w, 0, -1))  # [ci(,k), co]
        return np.ascontiguousarray(
            wt.reshape(CT, P, *wt.shape[1:]).transpose(1, 0, *range(2, wt.ndim + 1)))

    W1f = (W1 * np.asarray(g1)[None, :]).astype(f)      # fold gamma1
    b1f = (b1 + W1 @ be1).astype(f)                     # fold beta1
    r1 = (be1 + b2).astype(f)                           # residual additive fix
    flags = (not np.allclose(g1, 1.0), not np.allclose(r1, 0.0),
             not np.allclose(g2, 1.0), not np.allclose(be2, 0.0))

    def ctile_o(w):  # [co, ci, k] -> [P, co_t, ci_t, k, 128] (ct_out-major)
        a = ctile(w)                                     # [P, CT, KS, D]
        return np.ascontiguousarray(
            a.reshape(P, CT, KS, CT, P).transpose(0, 3, 1, 2, 4))

    pieces = {
        "wq": _bf16_pack(ctile_o(Wq)),                   # [P, CT, CT, KS, 128]
        "wk": _bf16_pack(ctile_o(Wk)),
        "wv": _bf16_pack(ctile(Wv[:, :, 0])),            # [P, CT, D]
        "wo": np.ascontiguousarray(
            Wo.T.reshape(CT, P, D).transpose(1, 0, 2)).astype(f),
        "w1": _bf16_pack(np.ascontiguousarray(
            W1f.T.reshape(CT, P, DFF).transpose(1, 0, 2))),
        "w2": _bf16_pack(np.ascontiguousarray(
            W2.T.reshape(FT, P, D).transpose(1, 0, 2))),
        "bq": np.ascontiguousarray(bq.reshape(CT, P).T).astype(f),
        "bk": np.ascontiguousarray(bk.reshape(CT, P).T).astype(f),
        "b1": np.ascontiguousarray(b1f.reshape(FT, P).T).astype(f),
        "ident": np.eye(P, dtype=f),
    }
    if flags[0]:
        pieces["g1r"] = np.ascontiguousarray(np.broadcast_to(g1, (P, D))).astype(f)
    if flags[1]:
        pieces["r1r"] = np.ascontiguousarray(np.broadcast_to(r1, (P, D))).astype(f)
    if flags[2]:
        pieces["g2r"] = np.ascontiguousarray(np.broadcast_to(g2, (P, D))).astype(f)
    if flags[3]:
        pieces["be2r"] = np.ascontiguousarray(np.broadcast_to(be2, (P, D))).astype(f)

    off, total = _blob_layout(flags)
    shared = np.zeros((P, total), f)
    for name, (o, ln) in off.items():
        if name in ("srcT", "src_sd"):
            continue
        shared[:, o:o + ln] = pieces[name].reshape(P, ln)

    bo2 = (bo + Wo @ bv).astype(f)                       # folded into residual
    o_srcT, l_srcT = off["srcT"]
    o_ssd, l_ssd = off["src_sd"]
    in_maps = []
    for b in range(NCORES):
        m = shared.copy()
        m[:, o_srcT:o_srcT + l_srcT] = _bf16_pack(np.ascontiguousarray(
            src[b].T.reshape(CT, P, S).transpose(1, 0, 2)))
        m[:, o_ssd:o_ssd + l_ssd] = np.ascontiguousarray(
            (src[b] + bo2[None, :]).reshape(ST, P, D).transpose(1, 0, 2)
        ).astype(f).reshape(P, l_ssd)
        in_maps.append({"blob": m})
    return in_maps, flags


def get_nc(flags=(False, False, False, False)):
    if ("nc", flags) not in _CACHE:
        _CACHE[("nc", flags)] = _build_nc(flags)
    return _CACHE[("nc", flags)]


def _get_runner(nc):
    """Cached jit(shard_map(bass_exec)) executor: trace/compile once, then
    each kernel() call is device_put + execute (run_bass_kernel_spmd
    rebuilds its jit closure every call, paying a full re-trace)."""
    key = ("runner", id(nc))
    if key in _CACHE:
        return _CACHE[key]
    import jax
    from jax.sharding import Mesh, PartitionSpec, NamedSharding
    try:
        from jax.shard_map import shard_map
    except ImportError:
        from jax.experimental.shard_map import shard_map
    from concourse import bass2jax, mybir

    bass2jax.install_neuronx_cc_hook()
    out_shape = (P, ST, D)
    out_avals = (jax.core.ShapedArray(out_shape, np.float32),)

    def _body(blob_in, out_zero):
        outs = bass2jax._bass_exec_p.bind(
            blob_in, out_zero,
            out_avals=out_avals,
            in_names=("blob", "out"),
            out_names=("out",),
            lowering_input_output_aliases=(),
            sim_require_finite=True,
            sim_require_nnan=True,
            nc=nc,
        )
        return tuple(outs)

    devices = jax.devices()[:NCORES]
    mesh = Mesh(np.asarray(devices), ("core",))
    spec = PartitionSpec("core")
    f = jax.jit(
        shard_map(_body, mesh=mesh, in_specs=(spec, spec), out_specs=(spec,),
                  check_rep=False),
        keep_unused=True,
    )
    sharding = NamedSharding(mesh, spec)
    zeros = np.zeros((NCORES * P, ST, D), np.float32)
    _CACHE[key] = (f, sharding, zeros, jax)
    return _CACHE[key]


def kernel(**inputs):
    in_maps, flags = _prep_inputs(**{k: np.asarray(v) for k, v in inputs.items()})
    nc = get_nc(flags)
    try:
        f, sharding, zeros, jax = _get_runner(nc)
        blob_all = np.concatenate([m["blob"] for m in in_maps], axis=0)
        blob_dev = jax.device_put(blob_all, sharding)
        out_dev = jax.device_put(zeros, sharding)
        (out_all,) = f(blob_dev, out_dev)
        out_np = np.asarray(out_all).reshape(NCORES, P, ST, D)
        outs = [out_np[c].transpose(1, 0, 2).reshape(S, D)
                for c in range(NCORES)]
        return np.stack(outs).astype(np.float32)
    except Exception:
        from concourse.bass_utils import run_bass_kernel_spmd
        res = run_bass_kernel_spmd(nc, in_maps, core_ids=list(range(NCORES)))
        outs = [r["out"].transpose(1, 0, 2).reshape(S, D) for r in res.results]
        return np.stack(outs).astype(np.float32)
